# revision 1
# baseline (speedup 1.0000x reference)
"""Trainium2 Bass kernel for nn_CVAEEncoder (2x GraphConv + concat + 2 dense heads).

Self-contained: hardcodes shapes/sharding for the 100k-node / 600k-edge problem.
Distributes over 8 NeuronCores: nodes sharded by id; edges partitioned by
receiver; sender features exchanged via AllGather of the per-layer Z tensor.

Scatter (segment_sum) implementation: edges grouped by (receiver-tile,
sender-subtable), gathered in bulk with dma_gather (int16 indices into one of
4 row-subtables of the AllGathered Z), re-aligned to receivers with one-hot
selector matrices built in one DVE op each (iota == r_e, scaled by
rsqrt(receiver_deg)), and accumulated on the TensorEngine into PSUM.
"""
import sys
sys.path.insert(0, "/opt/trn_rl_repo")
import numpy as np

N_NODES = 100000
N_EDGES = 600000
F = 128
ZDIM = 64
C = 8              # cores
RPC = N_NODES // C           # real nodes per core (12500)
T = (RPC + 127) // 128       # receiver tiles per core (98)
RPCP = T * 128               # padded nodes per core (12544)
VFULL = C * RPCP             # gathered table rows (100352)
NSUB = 4                     # int16 subtables
SUB = VFULL // NSUB          # 25088 rows per subtable
GSZ = 6                      # tiles per psum group (6 scatter psum banks + 2 dense)
MAX_CH_PER_CALL = 8          # <=1024 indices per dma_gather call

_CACHE = {}


def _plan_edges(senders, receivers):
    """Build the uniform-across-cores scatter plan + per-core panels."""
    senders = np.asarray(senders).astype(np.int64)
    receivers = np.asarray(receivers).astype(np.int64)
    deg_send = np.bincount(senders, minlength=N_NODES)
    deg_recv = np.bincount(receivers, minlength=N_NODES)
    rs_send_full = (1.0 / np.sqrt(np.maximum(deg_send, 1))).astype(np.float32)
    rr_full = (1.0 / np.sqrt(np.maximum(deg_recv, 1))).astype(np.float32)
    gslot = (senders // RPC) * RPCP + (senders % RPC)   # [E] sender global slot

    cores = []
    for c in range(C):
        m = (receivers >= c * RPC) & (receivers < (c + 1) * RPC)
        r_loc = (receivers[m] - c * RPC).astype(np.int64)
        gs = gslot[m]
        t = r_loc >> 7
        j = gs // SUB
        order = np.lexsort((r_loc, j, t))
        cores.append(dict(
            r_loc=r_loc[order], gs=gs[order], t=t[order], j=j[order],
            rr=rr_full[receivers[m][order]],
        ))

    # edges-per-(t,j) per core -> uniform chunk counts
    ecounts = np.zeros((C, T, NSUB), dtype=np.int64)
    for c in range(C):
        tj = cores[c]["t"] * NSUB + cores[c]["j"]
        cnt = np.bincount(tj, minlength=T * NSUB)
        ecounts[c] = cnt.reshape(T, NSUB)
    n_ch = np.maximum(1, -(-ecounts.max(axis=0) // 128))   # [T, NSUB]

    # build call/chunk schedule (shared across cores)
    # order: for tile-group g (GSZ tiles): for j: tiles in g: chunks
    calls = []      # dict(j, chunks=[(t, col, start, stop)], n_chunks)
    total_chunks = int(n_ch.sum())
    col = 0
    chunk_cols = {}   # (t, j, k) -> panel col
    last_of_tile = {}  # t -> (j,k) of final chunk
    first_of_tile = {}
    for t in range(T):
        ks = [(j, k) for j in range(NSUB) for k in range(int(n_ch[t, j]))]
        first_of_tile[t] = ks[0]
        last_of_tile[t] = ks[-1]
    n_groups = -(-T // GSZ)
    for g in range(n_groups):
        tiles = list(range(g * GSZ, min((g + 1) * GSZ, T)))
        for j in range(NSUB):
            pend = []
            for t in tiles:
                for k in range(int(n_ch[t, j])):
                    chunk_cols[(t, j, k)] = col
                    pend.append((t, j, k, col))
                    col += 1
            for i0 in range(0, len(pend), MAX_CH_PER_CALL):
                grpch = pend[i0:i0 + MAX_CH_PER_CALL]
                calls.append(dict(j=j, chunks=grpch))
    assert col == total_chunks

    # per-core panels
    ni_cols_total = sum(len(cl["chunks"]) * 8 for cl in calls)  # 128 idx -> 8 int16 cols
    idx_panels = np.zeros((C, 128, ni_cols_total), dtype=np.int16)
    re_panels = np.full((C, 128, total_chunks), -1.0, dtype=np.float32)
    s_panels = np.zeros((C, 128, total_chunks), dtype=np.float32)

    for c in range(C):
        d = cores[c]
        # run boundaries per (t, j)
        starts = np.zeros((T, NSUB), dtype=np.int64)
        np.cumsum(ecounts[c].reshape(-1), out=starts.reshape(-1)[:])
        starts = np.concatenate([[0], starts.reshape(-1)[:-1]]).reshape(T, NSUB)
        colptr = 0
        for cl in calls:
            jj = cl["j"]
            ni = len(cl["chunks"]) * 128
            vals = np.zeros(ni, dtype=np.int16)
            for ci, (t, j, k, pcol) in enumerate(cl["chunks"]):
                a = starts[t, j] + 128 * k
                b = min(starts[t, j] + ecounts[c][t, j], a + 128)
                n_e = max(0, b - a)
                sl = slice(ci * 128, ci * 128 + 128)
                if n_e > 0:
                    loc = (d["gs"][a:b] - j * SUB).astype(np.int16)
                    fill = np.empty(128, dtype=np.int16)
                    fill[:n_e] = loc
                    fill[n_e:] = loc[-1] if n_e else 0
                    vals[sl] = fill
                    re_panels[c, :n_e, pcol] = (d["r_loc"][a:b] & 127).astype(np.float32)
                    s_panels[c, :n_e, pcol] = d["rr"][a:b]
                else:
                    vals[sl] = 0   # gather row 0 of subtable; selector kills it
            # wrap: idx m at [m%16, m//16], tiled to 128 partitions
            wrap = np.zeros((16, ni // 16), dtype=np.int16)
            ar = np.arange(ni)
            wrap[ar % 16, ar // 16] = vals
            idx_panels[c, :, colptr:colptr + ni // 16] = np.tile(wrap, (8, 1))
            colptr += ni // 16

    rs_send_pc = np.zeros((C, 128, T), dtype=np.float32)
    for c in range(C):
        v = np.zeros(RPCP, dtype=np.float32)
        v[:RPC] = rs_send_full[c * RPC:(c + 1) * RPC]
        rs_send_pc[c] = v.reshape(T, 128).T

    return dict(calls=calls, total_chunks=total_chunks,
                first_of_tile=first_of_tile, last_of_tile=last_of_tile,
                chunk_cols=chunk_cols, ni_cols_total=ni_cols_total,
                idx_panels=idx_panels, re_panels=re_panels, s_panels=s_panels,
                rs_send_pc=rs_send_pc)


def _build_program(plan):
    from concourse import bass, bacc, mybir, tile
    from concourse.library_config import mlp
    f32 = mybir.dt.float32
    i16 = mybir.dt.int16

    nc = bacc.Bacc("TRN2", target_bir_lowering=False, debug=False,
                   num_devices=C, num_swdge_queues=4)

    xT_d = nc.dram_tensor("xT", [F, RPCP], f32, kind="ExternalInput").ap()
    W0_d = nc.dram_tensor("W0c", [F, F], f32, kind="ExternalInput").ap()
    b0_d = nc.dram_tensor("b0r", [1, F], f32, kind="ExternalInput").ap()
    W1_d = nc.dram_tensor("W1c", [F, F], f32, kind="ExternalInput").ap()
    b1_d = nc.dram_tensor("b1r", [1, F], f32, kind="ExternalInput").ap()
    rhsA_d = nc.dram_tensor("rhsA", [F, F], f32, kind="ExternalInput").ap()
    rhsB_d = nc.dram_tensor("rhsB", [F, F], f32, kind="ExternalInput").ap()
    bAB_d = nc.dram_tensor("bABr", [1, F], f32, kind="ExternalInput").ap()
    rs_d = nc.dram_tensor("rs_send", [128, T], f32, kind="ExternalInput").ap()
    iota_d = nc.dram_tensor("iota", [128, 128], f32, kind="ExternalInput").ap()
    idx_d = nc.dram_tensor("idxp", [128, plan["ni_cols_total"]], i16, kind="ExternalInput").ap()
    re_d = nc.dram_tensor("rep", [128, plan["total_chunks"]], f32, kind="ExternalInput").ap()
    s_d = nc.dram_tensor("sp", [128, plan["total_chunks"]], f32, kind="ExternalInput").ap()
    out_d = nc.dram_tensor("out", [RPCP, F], f32, kind="ExternalOutput").ap()

    with tile.TileContext(nc) as tc:
        with (
            tc.tile_pool(name="const", bufs=1) as constp,
            tc.tile_pool(name="panels", bufs=1) as panelp,
            tc.tile_pool(name="work", bufs=3) as work,
            tc.tile_pool(name="gp", bufs=4) as gp,
            tc.tile_pool(name="selp", bufs=4) as selp,
            tc.tile_pool(name="psum", bufs=1, space="PSUM") as psum,
            tc.tile_pool(name="dram", bufs=1, space="DRAM") as dram,
        ):
            nc.gpsimd.load_library(mlp)

            # ---- constants ----
            W0_sb = constp.tile([F, F], f32)
            W1_sb = constp.tile([F, F], f32)
            rhsA_sb = constp.tile([F, F], f32)
            rhsB_sb = constp.tile([F, F], f32)
            b0_sb = constp.tile([1, F], f32)
            b1_sb = constp.tile([1, F], f32)
            bAB_sb = constp.tile([1, F], f32)
            rs_sb = constp.tile([128, T], f32)
            iota_sb = constp.tile([128, 128], f32)
            ones_sb = constp.tile([1, F], f32)
            nc.sync.dma_start(out=W0_sb[:], in_=W0_d[:])
            nc.sync.dma_start(out=W1_sb[:], in_=W1_d[:])
            nc.sync.dma_start(out=rhsA_sb[:], in_=rhsA_d[:])
            nc.sync.dma_start(out=rhsB_sb[:], in_=rhsB_d[:])
            nc.sync.dma_start(out=b0_sb[:], in_=b0_d[:])
            nc.sync.dma_start(out=b1_sb[:], in_=b1_d[:])
            nc.sync.dma_start(out=bAB_sb[:], in_=bAB_d[:])
            nc.sync.dma_start(out=rs_sb[:], in_=rs_d[:])
            nc.sync.dma_start(out=iota_sb[:], in_=iota_d[:])
            nc.vector.memset(ones_sb[:], 1.0)

            idx_sb = panelp.tile([128, plan["ni_cols_total"]], i16)
            re_sb = panelp.tile([128, plan["total_chunks"]], f32)
            s_sb = panelp.tile([128, plan["total_chunks"]], f32)
            nc.sync.dma_start(out=idx_sb[:], in_=idx_d[:])
            nc.sync.dma_start(out=re_sb[:], in_=re_d[:])
            nc.sync.dma_start(out=s_sb[:], in_=s_d[:])

            xT_sb = panelp.tile([F, RPCP], f32)
            for q in range(4):
                w = RPCP // 4
                nc.sync.dma_start(out=xT_sb[:, q * w:(q + 1) * w],
                                  in_=xT_d[:, q * w:(q + 1) * w])
            hT_sb = panelp.tile([F, RPCP], f32)

            z1_shard = dram.tile([RPCP, F], f32)
            z2_shard = dram.tile([RPCP, F], f32)
            z1_full = dram.tile([VFULL, F], f32, addr_space="Shared")
            z2_full = dram.tile([VFULL, F], f32, addr_space="Shared")

            def dense_softmax(lhsT_panel, W_sb, b_sb, z_shard, tag):
                for t in range(T):
                    ps = psum.tile([128, F], f32, space="PSUM", tag="mm", bufs=2,
                                   name=f"ps_{tag}_{t}")
                    nc.tensor.matmul(out=ps[:], lhsT=lhsT_panel[:, t * 128:(t + 1) * 128],
                                     rhs=W_sb[:], start=True, stop=False)
                    nc.tensor.matmul(out=ps[:], lhsT=ones_sb[:], rhs=b_sb[:],
                                     start=False, stop=True)
                    relu_t = work.tile([128, F], f32, tag="relu", name=f"relu_{tag}_{t}")
                    nc.scalar.activation(out=relu_t[:], in_=ps[:],
                                         func=mybir.ActivationFunctionType.Relu)
                    expz = work.tile([128, F], f32, tag="expz", name=f"expz_{tag}_{t}")
                    sums = work.tile([128, 1], f32, tag="sums", name=f"sums_{tag}_{t}")
                    nc.scalar.activation(out=expz[:], in_=relu_t[:],
                                         func=mybir.ActivationFunctionType.Exp,
                                         accum_out=sums[:])
                    rec = work.tile([128, 1], f32, tag="rec", name=f"rec_{tag}_{t}")
                    nc.vector.reciprocal(out=rec[:], in_=sums[:])
                    z_t = work.tile([128, F], f32, tag="zt", name=f"z_{tag}_{t}")
                    nc.vector.tensor_scalar(out=z_t[:], in0=expz[:],
                                            scalar1=rec[:], scalar2=rs_sb[:, t:t + 1],
                                            op0=mybir.AluOpType.mult,
                                            op1=mybir.AluOpType.mult)
                    nc.sync.dma_start(out=z_shard[t * 128:(t + 1) * 128, :], in_=z_t[:])

            def scatter(z_full, tag):
                idx_off = 0
                psum_tiles = {}
                for ci, cl in enumerate(plan["calls"]):
                    nchk = len(cl["chunks"])
                    ni = nchk * 128
                    G = gp.tile([128, MAX_CH_PER_CALL, 128], f32, tag="G",
                                name=f"G_{tag}_{ci}")
                    jj = cl["j"]
                    nc.gpsimd.dma_gather(
                        G[:, :nchk, :],
                        z_full[jj * SUB:(jj + 1) * SUB, :],
                        idx_sb[:, idx_off:idx_off + ni // 16],
                        ni, ni, F,
                        queue_num=ci % 4,
                    )
                    idx_off += ni // 16
                    for bi, (t, j, k, pcol) in enumerate(cl["chunks"]):
                        sel = selp.tile([128, 128], f32, tag="sel",
                                        name=f"sel_{tag}_{pcol}")
                        nc.vector.tensor_scalar(
                            out=sel[:], in0=iota_sb[:],
                            scalar1=re_sb[:, pcol:pcol + 1],
                            scalar2=s_sb[:, pcol:pcol + 1],
                            op0=mybir.AluOpType.is_equal,
                            op1=mybir.AluOpType.mult)
                        first = plan["first_of_tile"][t] == (j, k)
                        last = plan["last_of_tile"][t] == (j, k)
                        if first:
                            psum_tiles[t] = psum.tile([128, 128], f32, space="PSUM",
                                                      tag="sc", bufs=6,
                                                      name=f"psc_{tag}_{t}")
                        nc.tensor.matmul(out=psum_tiles[t][:], lhsT=G[:, bi, :],
                                         rhs=sel[:], start=first, stop=last)
                        if last:
                            nc.scalar.activation(
                                out=hT_sb[:, t * 128:(t + 1) * 128],
                                in_=psum_tiles[t][:],
                                func=mybir.ActivationFunctionType.Copy)
                            del psum_tiles[t]

            # ---- layer 1 ----
            dense_softmax(xT_sb, W0_sb, b0_sb, z1_shard, "l1")
            nc.gpsimd.collective_compute(
                "AllGather", mybir.AluOpType.bypass,
                replica_groups=[list(range(C))],
                ins=[z1_shard.opt()], outs=[z1_full.opt()])
            scatter(z1_full, "s1")
            # ---- layer 2 ----
            dense_softmax(hT_sb, W1_sb, b1_sb, z2_shard, "l2")
            nc.gpsimd.collective_compute(
                "AllGather", mybir.AluOpType.bypass,
                replica_groups=[list(range(C))],
                ins=[z2_shard.opt()], outs=[z2_full.opt()])
            scatter(z2_full, "s2")
            # ---- heads ----
            for t in range(T):
                ps = psum.tile([128, F], f32, space="PSUM", tag="mm", bufs=2,
                               name=f"ps_fin_{t}")
                nc.tensor.matmul(out=ps[:], lhsT=hT_sb[:, t * 128:(t + 1) * 128],
                                 rhs=rhsA_sb[:], start=True, stop=False)
                nc.tensor.matmul(out=ps[:], lhsT=xT_sb[:, t * 128:(t + 1) * 128],
                                 rhs=rhsB_sb[:], start=False, stop=False)
                nc.tensor.matmul(out=ps[:], lhsT=ones_sb[:], rhs=bAB_sb[:],
                                 start=False, stop=True)
                o_t = work.tile([128, F], f32, tag="ot", name=f"o_{t}")
                nc.scalar.activation(out=o_t[:], in_=ps[:],
                                     func=mybir.ActivationFunctionType.Copy)
                nc.sync.dma_start(out=out_d[t * 128:(t + 1) * 128, :], in_=o_t[:])

    nc.compile()
    return nc


def _get_runner(senders, receivers):
    key = (int(np.asarray(senders)[:3].sum()), int(np.asarray(receivers)[:3].sum()),
           len(np.asarray(senders)))
    if key in _CACHE:
        return _CACHE[key]
    plan = _plan_edges(senders, receivers)
    nc = _build_program(plan)
    sys.path.insert(0, "/root/problem")
    try:
        from pjrt_runner import Runner
    except ImportError:
        Runner = _inline_runner()
    runner = Runner(nc, C)
    _CACHE[key] = (plan, runner)
    return plan, runner


def _inline_runner():
    """Minimal copy of the Runner used when pjrt_runner.py isn't present."""
    import jax
    from jax.sharding import Mesh, PartitionSpec
    from jax.experimental.shard_map import shard_map
    from concourse import mybir
    from concourse.bass2jax import (_bass_exec_p, partition_id_tensor,
                                    install_neuronx_cc_hook)

    class Runner:
        def __init__(self, nc, n_cores):
            install_neuronx_cc_hook()
            self.nc, self.n_cores = nc, n_cores
            pname = nc.partition_id_tensor.name if nc.partition_id_tensor else None
            self.in_names, self.out_names, self.out_avals, self.zero_outs = [], [], [], []
            for alloc in nc.m.functions[0].allocations:
                if not isinstance(alloc, mybir.MemoryLocationSet):
                    continue
                name = alloc.memorylocations[0].name
                if alloc.kind == "ExternalInput":
                    if name != pname:
                        self.in_names.append(name)
                elif alloc.kind == "ExternalOutput":
                    shape = tuple(alloc.tensor_shape)
                    dtype = mybir.dt.np(alloc.dtype)
                    self.out_names.append(name)
                    self.out_avals.append(jax.core.ShapedArray(shape, dtype))
                    self.zero_outs.append(np.zeros(shape, dtype))
            n_params, n_outs = len(self.in_names), len(self.out_avals)
            self.n_params = n_params
            all_in = self.in_names + self.out_names + ([pname] if pname else [])
            donate = tuple(range(n_params, n_params + n_outs))
            out_avals, out_names = self.out_avals, self.out_names

            def _body(*args):
                operands = list(args)
                if pname is not None:
                    operands.append(partition_id_tensor())
                return tuple(_bass_exec_p.bind(
                    *operands, out_avals=tuple(out_avals),
                    in_names=tuple(all_in), out_names=tuple(out_names),
                    lowering_input_output_aliases=(),
                    sim_require_finite=True, sim_require_nnan=True, nc=nc))

            devices = jax.devices()[:n_cores]
            self.mesh = Mesh(np.asarray(devices), ("core",))
            in_specs = (PartitionSpec("core"),) * (n_params + n_outs)
            out_specs = (PartitionSpec("core"),) * n_outs
            self.sharded = jax.jit(
                shard_map(_body, mesh=self.mesh, in_specs=in_specs,
                          out_specs=out_specs, check_rep=False),
                donate_argnums=donate, keep_unused=True)
            self.sharding = jax.sharding.NamedSharding(self.mesh, PartitionSpec("core"))

        def stage_inputs(self, in_maps):
            import jax
            per_core = [[np.asarray(m[n]) for n in self.in_names] for m in in_maps]
            concat = [np.ascontiguousarray(np.concatenate(
                [per_core[c][i] for c in range(self.n_cores)], axis=0))
                for i in range(self.n_params)]
            return [jax.device_put(a, self.sharding) for a in concat]

        def stage_zeros(self):
            import jax
            return [jax.device_put(np.zeros((self.n_cores * z.shape[0], *z.shape[1:]),
                                            z.dtype), self.sharding)
                    for z in self.zero_outs]

        def run_staged(self, dev_in, dev_zeros):
            import jax
            out = self.sharded(*dev_in, *dev_zeros)
            jax.block_until_ready(out)
            return out

        def run(self, in_maps):
            out = self.run_staged(self.stage_inputs(in_maps), self.stage_zeros())
            res = []
            for c in range(self.n_cores):
                res.append({n: np.asarray(out[i]).reshape(
                    self.n_cores, *self.out_avals[i].shape)[c]
                    for i, n in enumerate(self.out_names)})
            return res

    return Runner


def _make_in_maps(plan, nodes, W0, b0, W1, b1, Wmu, bmu, Wls, bls):
    nodes = np.asarray(nodes, dtype=np.float32)
    rhsA = np.concatenate([np.asarray(Wmu)[:F, :], np.asarray(Wls)[:F, :]], axis=1).astype(np.float32)
    rhsB = np.concatenate([np.asarray(Wmu)[F:, :], np.asarray(Wls)[F:, :]], axis=1).astype(np.float32)
    bAB = np.concatenate([np.asarray(bmu), np.asarray(bls)])[None, :].astype(np.float32)
    iota = np.tile(np.arange(128, dtype=np.float32), (128, 1))
    in_maps = []
    for c in range(C):
        xc = np.zeros((RPCP, F), dtype=np.float32)
        xc[:RPC] = nodes[c * RPC:(c + 1) * RPC]
        in_maps.append(dict(
            xT=np.ascontiguousarray(xc.T),
            W0c=np.asarray(W0, dtype=np.float32),
            b0r=np.asarray(b0, dtype=np.float32)[None, :],
            W1c=np.asarray(W1, dtype=np.float32),
            b1r=np.asarray(b1, dtype=np.float32)[None, :],
            rhsA=rhsA, rhsB=rhsB, bABr=bAB,
            rs_send=plan["rs_send_pc"][c],
            iota=iota,
            idxp=plan["idx_panels"][c],
            rep=plan["re_panels"][c],
            sp=plan["s_panels"][c],
        ))
    return in_maps


def kernel(nodes, senders, receivers, W0, b0, W1, b1, Wmu, bmu, Wls, bls):
    plan, runner = _get_runner(senders, receivers)
    in_maps = _make_in_maps(plan, nodes, W0, b0, W1, b1, Wmu, bmu, Wls, bls)
    res = runner.run(in_maps)
    full = np.concatenate([res[c]["out"][:RPC] for c in range(C)], axis=0)
    mu = full[:, :ZDIM]
    logsig2 = full[:, ZDIM:]
    return (mu, logsig2)


# revision 5
# speedup vs baseline: 7.1240x; 7.1240x over previous
"""Trainium2 Bass kernel for nn_CVAEEncoder (2x GraphConv + concat + 2 dense heads).

Self-contained: hardcodes shapes/sharding for the 100k-node / 600k-edge problem.
Distributes over 8 NeuronCores: nodes sharded by id; edges partitioned by
receiver; sender features exchanged via AllGather of the per-layer Z tensor.

Scatter (segment_sum) implementation: edges grouped by (receiver-tile,
sender-subtable), gathered in bulk with dma_gather (int16 indices into one of
4 row-subtables of the AllGathered Z), re-aligned to receivers with one-hot
selector matrices built in one DVE op each (iota == r_e, scaled by
rsqrt(receiver_deg)), and accumulated on the TensorEngine into PSUM.
"""
import sys
sys.path.insert(0, "/opt/trn_rl_repo")
import numpy as np

N_NODES = 100000
N_EDGES = 600000
F = 128
ZDIM = 64
C = 8              # cores
RPC = N_NODES // C           # real nodes per core (12500)
T = (RPC + 127) // 128       # receiver tiles per core (98)
RPCP = T * 128               # padded nodes per core (12544)
VFULL = C * RPCP             # gathered table rows (100352)
NSUB = 4                     # int16 subtables
SUB = VFULL // NSUB          # 25088 rows per subtable
GSZ = 6                      # tiles per psum group (6 scatter psum banks + 2 dense)
MAX_CH_PER_CALL = 8          # <=1024 indices per dma_gather call

_CACHE = {}


def _plan_edges(senders, receivers):
    """Build the uniform-across-cores scatter plan + per-core panels."""
    senders = np.asarray(senders).astype(np.int64)
    receivers = np.asarray(receivers).astype(np.int64)
    deg_send = np.bincount(senders, minlength=N_NODES)
    deg_recv = np.bincount(receivers, minlength=N_NODES)
    rs_send_full = (1.0 / np.sqrt(np.maximum(deg_send, 1))).astype(np.float32)
    rr_full = (1.0 / np.sqrt(np.maximum(deg_recv, 1))).astype(np.float32)
    gslot = (senders // RPC) * RPCP + (senders % RPC)   # [E] sender global slot

    cores = []
    for c in range(C):
        m = (receivers >= c * RPC) & (receivers < (c + 1) * RPC)
        r_loc = (receivers[m] - c * RPC).astype(np.int64)
        gs = gslot[m]
        t = r_loc >> 7
        j = gs // SUB
        order = np.lexsort((r_loc, j, t))
        cores.append(dict(
            r_loc=r_loc[order], gs=gs[order], t=t[order], j=j[order],
            rr=rr_full[receivers[m][order]],
        ))

    # edges-per-(t,j) per core -> uniform chunk counts
    ecounts = np.zeros((C, T, NSUB), dtype=np.int64)
    for c in range(C):
        tj = cores[c]["t"] * NSUB + cores[c]["j"]
        cnt = np.bincount(tj, minlength=T * NSUB)
        ecounts[c] = cnt.reshape(T, NSUB)
    n_ch = np.maximum(1, -(-ecounts.max(axis=0) // 128))   # [T, NSUB]

    # build call/chunk schedule (shared across cores)
    # order: for tile-group g (GSZ tiles): for j: tiles in g: chunks
    calls = []      # dict(j, chunks=[(t, col, start, stop)], n_chunks)
    total_chunks = int(n_ch.sum())
    col = 0
    chunk_cols = {}   # (t, j, k) -> panel col
    last_of_tile = {}  # t -> (j,k) of final chunk
    first_of_tile = {}
    for t in range(T):
        ks = [(j, k) for j in range(NSUB) for k in range(int(n_ch[t, j]))]
        first_of_tile[t] = ks[0]
        last_of_tile[t] = ks[-1]
    n_groups = -(-T // GSZ)
    for g in range(n_groups):
        tiles = list(range(g * GSZ, min((g + 1) * GSZ, T)))
        for j in range(NSUB):
            pend = []
            for t in tiles:
                for k in range(int(n_ch[t, j])):
                    chunk_cols[(t, j, k)] = col
                    pend.append((t, j, k, col))
                    col += 1
            for i0 in range(0, len(pend), MAX_CH_PER_CALL):
                grpch = pend[i0:i0 + MAX_CH_PER_CALL]
                calls.append(dict(j=j, chunks=grpch))
    assert col == total_chunks

    # per-core panels
    ni_cols_total = sum(len(cl["chunks"]) * 8 for cl in calls)  # 128 idx -> 8 int16 cols
    idx_panels = np.zeros((C, 128, ni_cols_total), dtype=np.int16)
    re_panels = np.full((C, 128, total_chunks), -1.0, dtype=np.float32)
    s_panels = np.zeros((C, 128, total_chunks), dtype=np.float32)

    for c in range(C):
        d = cores[c]
        # run boundaries per (t, j)
        starts = np.zeros((T, NSUB), dtype=np.int64)
        np.cumsum(ecounts[c].reshape(-1), out=starts.reshape(-1)[:])
        starts = np.concatenate([[0], starts.reshape(-1)[:-1]]).reshape(T, NSUB)
        colptr = 0
        for cl in calls:
            jj = cl["j"]
            ni = len(cl["chunks"]) * 128
            vals = np.zeros(ni, dtype=np.int16)
            for ci, (t, j, k, pcol) in enumerate(cl["chunks"]):
                a = starts[t, j] + 128 * k
                b = min(starts[t, j] + ecounts[c][t, j], a + 128)
                n_e = max(0, b - a)
                sl = slice(ci * 128, ci * 128 + 128)
                if n_e > 0:
                    loc = (d["gs"][a:b] - j * SUB).astype(np.int16)
                    fill = np.empty(128, dtype=np.int16)
                    fill[:n_e] = loc
                    fill[n_e:] = loc[-1] if n_e else 0
                    vals[sl] = fill
                    re_panels[c, :n_e, pcol] = (d["r_loc"][a:b] & 127).astype(np.float32)
                    s_panels[c, :n_e, pcol] = d["rr"][a:b]
                else:
                    vals[sl] = 0   # gather row 0 of subtable; selector kills it
            # wrap: idx m at [m%16, m//16], tiled to 128 partitions
            wrap = np.zeros((16, ni // 16), dtype=np.int16)
            ar = np.arange(ni)
            wrap[ar % 16, ar // 16] = vals
            idx_panels[c, :, colptr:colptr + ni // 16] = np.tile(wrap, (8, 1))
            colptr += ni // 16

    rs_send_pc = np.zeros((C, 128, T), dtype=np.float32)
    for c in range(C):
        v = np.zeros(RPCP, dtype=np.float32)
        v[:RPC] = rs_send_full[c * RPC:(c + 1) * RPC]
        rs_send_pc[c] = v.reshape(T, 128).T

    return dict(calls=calls, total_chunks=total_chunks,
                first_of_tile=first_of_tile, last_of_tile=last_of_tile,
                chunk_cols=chunk_cols, ni_cols_total=ni_cols_total,
                idx_panels=idx_panels, re_panels=re_panels, s_panels=s_panels,
                rs_send_pc=rs_send_pc)


def _build_program(plan, phases="full", reps=1):
    from concourse import bass, bacc, mybir, tile
    from concourse.library_config import mlp
    f32 = mybir.dt.float32
    i16 = mybir.dt.int16

    nc = bacc.Bacc("TRN2", target_bir_lowering=False, debug=False,
                   num_devices=C, num_swdge_queues=4)

    xT_d = nc.dram_tensor("xT", [F, RPCP], f32, kind="ExternalInput").ap()
    W0_d = nc.dram_tensor("W0c", [F, F], f32, kind="ExternalInput").ap()
    b0_d = nc.dram_tensor("b0r", [1, F], f32, kind="ExternalInput").ap()
    W1_d = nc.dram_tensor("W1c", [F, F], f32, kind="ExternalInput").ap()
    b1_d = nc.dram_tensor("b1r", [1, F], f32, kind="ExternalInput").ap()
    rhsA_d = nc.dram_tensor("rhsA", [F, F], f32, kind="ExternalInput").ap()
    rhsB_d = nc.dram_tensor("rhsB", [F, F], f32, kind="ExternalInput").ap()
    bAB_d = nc.dram_tensor("bABr", [1, F], f32, kind="ExternalInput").ap()
    rs_d = nc.dram_tensor("rs_send", [128, T], f32, kind="ExternalInput").ap()
    iota_d = nc.dram_tensor("iota", [128, 128], f32, kind="ExternalInput").ap()
    idx_d = nc.dram_tensor("idxp", [128, plan["ni_cols_total"]], i16, kind="ExternalInput").ap()
    re_d = nc.dram_tensor("rep", [128, plan["total_chunks"]], f32, kind="ExternalInput").ap()
    s_d = nc.dram_tensor("sp", [128, plan["total_chunks"]], f32, kind="ExternalInput").ap()
    out_d = nc.dram_tensor("out", [RPCP, F], f32, kind="ExternalOutput").ap()

    with tile.TileContext(nc) as tc:
        with (
            tc.tile_pool(name="const", bufs=1) as constp,
            tc.tile_pool(name="panels", bufs=1) as panelp,
            tc.tile_pool(name="work", bufs=3) as work,
            tc.tile_pool(name="gp", bufs=4) as gp,
            tc.tile_pool(name="selp", bufs=4) as selp,
            tc.tile_pool(name="psum", bufs=1, space="PSUM") as psum,
            tc.tile_pool(name="dram", bufs=1, space="DRAM") as dram,
        ):
            nc.gpsimd.load_library(mlp)

            # ---- constants ----
            W0_sb = constp.tile([F, F], f32)
            W1_sb = constp.tile([F, F], f32)
            rhsA_sb = constp.tile([F, F], f32)
            rhsB_sb = constp.tile([F, F], f32)
            b0_sb = constp.tile([1, F], f32)
            b1_sb = constp.tile([1, F], f32)
            bAB_sb = constp.tile([1, F], f32)
            rs_sb = constp.tile([128, T], f32)
            iota_sb = constp.tile([128, 128], f32)
            ones_sb = constp.tile([1, F], f32)
            nc.sync.dma_start(out=W0_sb[:], in_=W0_d[:])
            nc.sync.dma_start(out=W1_sb[:], in_=W1_d[:])
            nc.sync.dma_start(out=rhsA_sb[:], in_=rhsA_d[:])
            nc.sync.dma_start(out=rhsB_sb[:], in_=rhsB_d[:])
            nc.sync.dma_start(out=b0_sb[:], in_=b0_d[:])
            nc.sync.dma_start(out=b1_sb[:], in_=b1_d[:])
            nc.sync.dma_start(out=bAB_sb[:], in_=bAB_d[:])
            nc.sync.dma_start(out=rs_sb[:], in_=rs_d[:])
            nc.sync.dma_start(out=iota_sb[:], in_=iota_d[:])
            nc.vector.memset(ones_sb[:], 1.0)

            idx_sb = panelp.tile([128, plan["ni_cols_total"]], i16)
            re_sb = panelp.tile([128, plan["total_chunks"]], f32)
            s_sb = panelp.tile([128, plan["total_chunks"]], f32)
            nc.sync.dma_start(out=idx_sb[:], in_=idx_d[:])
            nc.sync.dma_start(out=re_sb[:], in_=re_d[:])
            nc.sync.dma_start(out=s_sb[:], in_=s_d[:])

            xT_sb = panelp.tile([F, RPCP], f32)
            for q in range(4):
                w = RPCP // 4
                nc.sync.dma_start(out=xT_sb[:, q * w:(q + 1) * w],
                                  in_=xT_d[:, q * w:(q + 1) * w])
            hT_sb = panelp.tile([F, RPCP], f32)

            z1_shard = dram.tile([RPCP, F], f32)
            z2_shard = dram.tile([RPCP, F], f32)
            z1_full = dram.tile([VFULL, F], f32, addr_space="Shared")
            z2_full = dram.tile([VFULL, F], f32, addr_space="Shared")

            def dense_softmax(lhsT_panel, W_sb, b_sb, z_shard, tag):
                for t in range(T):
                    ps = psum.tile([128, F], f32, space="PSUM", tag="mm", bufs=2,
                                   name=f"ps_{tag}_{t}")
                    nc.tensor.matmul(out=ps[:], lhsT=lhsT_panel[:, t * 128:(t + 1) * 128],
                                     rhs=W_sb[:], start=True, stop=False)
                    nc.tensor.matmul(out=ps[:], lhsT=ones_sb[:], rhs=b_sb[:],
                                     start=False, stop=True)
                    relu_t = work.tile([128, F], f32, tag="relu", name=f"relu_{tag}_{t}")
                    nc.scalar.activation(out=relu_t[:], in_=ps[:],
                                         func=mybir.ActivationFunctionType.Relu)
                    expz = work.tile([128, F], f32, tag="expz", name=f"expz_{tag}_{t}")
                    sums = work.tile([128, 1], f32, tag="sums", name=f"sums_{tag}_{t}")
                    nc.scalar.activation(out=expz[:], in_=relu_t[:],
                                         func=mybir.ActivationFunctionType.Exp,
                                         accum_out=sums[:])
                    rec = work.tile([128, 1], f32, tag="rec", name=f"rec_{tag}_{t}")
                    nc.vector.reciprocal(out=rec[:], in_=sums[:])
                    z_t = work.tile([128, F], f32, tag="zt", name=f"z_{tag}_{t}")
                    nc.vector.tensor_scalar(out=z_t[:], in0=expz[:],
                                            scalar1=rec[:], scalar2=rs_sb[:, t:t + 1],
                                            op0=mybir.AluOpType.mult,
                                            op1=mybir.AluOpType.mult)
                    nc.sync.dma_start(out=z_shard[t * 128:(t + 1) * 128, :], in_=z_t[:])

            def scatter(z_full, tag):
                idx_off = 0
                psum_tiles = {}
                for ci, cl in enumerate(plan["calls"]):
                    nchk = len(cl["chunks"])
                    ni = nchk * 128
                    G = gp.tile([128, MAX_CH_PER_CALL, 128], f32, tag="G",
                                name=f"G_{tag}_{ci}")
                    jj = cl["j"]
                    nc.gpsimd.dma_gather(
                        G[:, :nchk, :],
                        z_full[jj * SUB:(jj + 1) * SUB, :],
                        idx_sb[:, idx_off:idx_off + ni // 16],
                        ni, ni, F,
                        queue_num=ci % 4,
                    )
                    idx_off += ni // 16
                    for bi, (t, j, k, pcol) in enumerate(cl["chunks"]):
                        sel = selp.tile([128, 128], f32, tag="sel",
                                        name=f"sel_{tag}_{pcol}")
                        nc.vector.tensor_scalar(
                            out=sel[:], in0=iota_sb[:],
                            scalar1=re_sb[:, pcol:pcol + 1],
                            scalar2=s_sb[:, pcol:pcol + 1],
                            op0=mybir.AluOpType.is_equal,
                            op1=mybir.AluOpType.mult)
                        first = plan["first_of_tile"][t] == (j, k)
                        last = plan["last_of_tile"][t] == (j, k)
                        if first:
                            psum_tiles[t] = psum.tile([128, 128], f32, space="PSUM",
                                                      tag="sc", bufs=6,
                                                      name=f"psc_{tag}_{t}")
                        nc.tensor.matmul(out=psum_tiles[t][:], lhsT=G[:, bi, :],
                                         rhs=sel[:], start=first, stop=last)
                        if last:
                            nc.scalar.activation(
                                out=hT_sb[:, t * 128:(t + 1) * 128],
                                in_=psum_tiles[t][:],
                                func=mybir.ActivationFunctionType.Copy)
                            del psum_tiles[t]

            # ---- layer 1 ----
            for _rep in range(reps):
              if phases != "null":
                dense_softmax(xT_sb, W0_sb, b0_sb, z1_shard, f"l1r{_rep}")
              if phases in ("d1+ag", "d1+ag+s1", "full"):
                nc.gpsimd.collective_compute(
                    "AllGather", mybir.AluOpType.bypass,
                    replica_groups=[list(range(C))],
                    ins=[z1_shard.opt()], outs=[z1_full.opt()])
              if phases in ("d1+ag+s1", "full"):
                scatter(z1_full, f"s1r{_rep}")
              else:
                nc.vector.memset(hT_sb[:], 0.0)
              if phases == "full":
                # ---- layer 2 ----
                dense_softmax(hT_sb, W1_sb, b1_sb, z2_shard, f"l2r{_rep}")
                nc.gpsimd.collective_compute(
                    "AllGather", mybir.AluOpType.bypass,
                    replica_groups=[list(range(C))],
                    ins=[z2_shard.opt()], outs=[z2_full.opt()])
                scatter(z2_full, f"s2r{_rep}")
            # ---- heads ----
            for t in range(T):
                ps = psum.tile([128, F], f32, space="PSUM", tag="mm", bufs=2,
                               name=f"ps_fin_{t}")
                nc.tensor.matmul(out=ps[:], lhsT=hT_sb[:, t * 128:(t + 1) * 128],
                                 rhs=rhsA_sb[:], start=True, stop=False)
                nc.tensor.matmul(out=ps[:], lhsT=xT_sb[:, t * 128:(t + 1) * 128],
                                 rhs=rhsB_sb[:], start=False, stop=False)
                nc.tensor.matmul(out=ps[:], lhsT=ones_sb[:], rhs=bAB_sb[:],
                                 start=False, stop=True)
                o_t = work.tile([128, F], f32, tag="ot", name=f"o_{t}")
                nc.scalar.activation(out=o_t[:], in_=ps[:],
                                     func=mybir.ActivationFunctionType.Copy)
                nc.sync.dma_start(out=out_d[t * 128:(t + 1) * 128, :], in_=o_t[:])

    nc.compile()
    return nc


def _get_runner(senders, receivers):
    key = (int(np.asarray(senders)[:3].sum()), int(np.asarray(receivers)[:3].sum()),
           len(np.asarray(senders)))
    if key in _CACHE:
        return _CACHE[key]
    plan = _plan_edges(senders, receivers)
    nc = _build_program(plan)
    sys.path.insert(0, "/root/problem")
    try:
        from pjrt_runner import Runner
    except ImportError:
        Runner = _inline_runner()
    runner = Runner(nc, C)
    _CACHE[key] = (plan, runner)
    return plan, runner


def _inline_runner():
    """Minimal copy of the Runner used when pjrt_runner.py isn't present."""
    import jax
    from jax.sharding import Mesh, PartitionSpec
    from jax.experimental.shard_map import shard_map
    from concourse import mybir
    from concourse.bass2jax import (_bass_exec_p, partition_id_tensor,
                                    install_neuronx_cc_hook)

    class Runner:
        def __init__(self, nc, n_cores):
            install_neuronx_cc_hook()
            self.nc, self.n_cores = nc, n_cores
            pname = nc.partition_id_tensor.name if nc.partition_id_tensor else None
            self.in_names, self.out_names, self.out_avals, self.zero_outs = [], [], [], []
            for alloc in nc.m.functions[0].allocations:
                if not isinstance(alloc, mybir.MemoryLocationSet):
                    continue
                name = alloc.memorylocations[0].name
                if alloc.kind == "ExternalInput":
                    if name != pname:
                        self.in_names.append(name)
                elif alloc.kind == "ExternalOutput":
                    shape = tuple(alloc.tensor_shape)
                    dtype = mybir.dt.np(alloc.dtype)
                    self.out_names.append(name)
                    self.out_avals.append(jax.core.ShapedArray(shape, dtype))
                    self.zero_outs.append(np.zeros(shape, dtype))
            n_params, n_outs = len(self.in_names), len(self.out_avals)
            self.n_params = n_params
            all_in = self.in_names + self.out_names + ([pname] if pname else [])
            donate = tuple(range(n_params, n_params + n_outs))
            out_avals, out_names = self.out_avals, self.out_names

            def _body(*args):
                operands = list(args)
                if pname is not None:
                    operands.append(partition_id_tensor())
                return tuple(_bass_exec_p.bind(
                    *operands, out_avals=tuple(out_avals),
                    in_names=tuple(all_in), out_names=tuple(out_names),
                    lowering_input_output_aliases=(),
                    sim_require_finite=True, sim_require_nnan=True, nc=nc))

            devices = jax.devices()[:n_cores]
            self.mesh = Mesh(np.asarray(devices), ("core",))
            in_specs = (PartitionSpec("core"),) * (n_params + n_outs)
            out_specs = (PartitionSpec("core"),) * n_outs
            self.sharded = jax.jit(
                shard_map(_body, mesh=self.mesh, in_specs=in_specs,
                          out_specs=out_specs, check_rep=False),
                donate_argnums=donate, keep_unused=True)
            self.sharding = jax.sharding.NamedSharding(self.mesh, PartitionSpec("core"))

        def stage_inputs(self, in_maps):
            import jax
            per_core = [[np.asarray(m[n]) for n in self.in_names] for m in in_maps]
            concat = [np.ascontiguousarray(np.concatenate(
                [per_core[c][i] for c in range(self.n_cores)], axis=0))
                for i in range(self.n_params)]
            return [jax.device_put(a, self.sharding) for a in concat]

        def stage_zeros(self):
            import jax
            return [jax.device_put(np.zeros((self.n_cores * z.shape[0], *z.shape[1:]),
                                            z.dtype), self.sharding)
                    for z in self.zero_outs]

        def run_staged(self, dev_in, dev_zeros):
            import jax
            out = self.sharded(*dev_in, *dev_zeros)
            jax.block_until_ready(out)
            return out

        def run(self, in_maps):
            out = self.run_staged(self.stage_inputs(in_maps), self.stage_zeros())
            res = []
            for c in range(self.n_cores):
                res.append({n: np.asarray(out[i]).reshape(
                    self.n_cores, *self.out_avals[i].shape)[c]
                    for i, n in enumerate(self.out_names)})
            return res

    return Runner


def _make_in_maps(plan, nodes, W0, b0, W1, b1, Wmu, bmu, Wls, bls):
    nodes = np.asarray(nodes, dtype=np.float32)
    rhsA = np.concatenate([np.asarray(Wmu)[:F, :], np.asarray(Wls)[:F, :]], axis=1).astype(np.float32)
    rhsB = np.concatenate([np.asarray(Wmu)[F:, :], np.asarray(Wls)[F:, :]], axis=1).astype(np.float32)
    bAB = np.concatenate([np.asarray(bmu), np.asarray(bls)])[None, :].astype(np.float32)
    iota = np.tile(np.arange(128, dtype=np.float32), (128, 1))
    in_maps = []
    for c in range(C):
        xc = np.zeros((RPCP, F), dtype=np.float32)
        xc[:RPC] = nodes[c * RPC:(c + 1) * RPC]
        in_maps.append(dict(
            xT=np.ascontiguousarray(xc.T),
            W0c=np.asarray(W0, dtype=np.float32),
            b0r=np.asarray(b0, dtype=np.float32)[None, :],
            W1c=np.asarray(W1, dtype=np.float32),
            b1r=np.asarray(b1, dtype=np.float32)[None, :],
            rhsA=rhsA, rhsB=rhsB, bABr=bAB,
            rs_send=plan["rs_send_pc"][c],
            iota=iota,
            idxp=plan["idx_panels"][c],
            rep=plan["re_panels"][c],
            sp=plan["s_panels"][c],
        ))
    return in_maps


def kernel(nodes, senders, receivers, W0, b0, W1, b1, Wmu, bmu, Wls, bls):
    plan, runner = _get_runner(senders, receivers)
    in_maps = _make_in_maps(plan, nodes, W0, b0, W1, b1, Wmu, bmu, Wls, bls)
    res = runner.run(in_maps)
    full = np.concatenate([res[c]["out"][:RPC] for c in range(C)], axis=0)
    mu = full[:, :ZDIM]
    logsig2 = full[:, ZDIM:]
    return (mu, logsig2)
